# revision 12
# baseline (speedup 1.0000x reference)
"""Graph-transformer block on 8 Trainium2 NeuronCores.

Sharding: each core takes a 512-row q-slice of the 4096 nodes across ALL 4
heads. No cross-core communication: each core computes attention for its
q rows, runs the FFN on its node slice, writes its [512, 256] output slice.

Per-core pipeline (transposed-S orientation — no on-device transposes):
  prep:  hT bf16; qT/kT per head pair packed on partitions; v with an
         interleaved all-ones column per head (the softmax denominator
         rides the PV matmul as output row 64). PSUM->SBUF prep copies go
         through the Scalar engine (ACT) which has slack; GPSIMD cannot
         read PSUM at all.
  attn:  per 128-wide j-block jb:
           one fp8 DMA of adjT for all 4 heads [128j, 4*512q]
           per head pair g: two S^T matmuls into one [128, 2, 512] PSUM
             tile (head 2g+i from partitions i*64 of the packed kT/qT)
           mk = (S^T * 1/16) * adjT   (DVE STT, PSUM fp32 * fp8 -> bf16)
           P^T = exp(mk) on ACT, one [128, 2048] activation per j-block
           xaug[hd] [65, 512] += v_aug_blk.T @ P^T on PE (PSUM accum)
         PV matmuls lag the S^T stage by PIPE j-blocks so the in-order PE
         stream never waits on the DVE->ACT chain.
  fin:   embT = xaug[0:64] * recip(xaug[64]) broadcast -> [256, 512] bf16
  ffn:   p1^T = relu(W1.T @ embT + b1); p2 = p1 @ W2 + b2 (natural [q,f])
         row softmax over 256 features; DMA out fp32
"""
import sys
import numpy as np

sys.path.insert(0, "/opt/trn_rl_repo")
import ml_dtypes  # noqa: E402

IN = 256
H = 4
DH = 64
NCORES = 8
F1 = 512
DOUT = 256
N_NODES = 4096
QS = 512
NJB = N_NODES // 128
SCALE = 1.0 / 16.0  # 1/sqrt(IN)
PIPE = 2            # PV matmuls lag S^T by this many j-blocks

_cache = {}


def build():
    if "nc" in _cache:
        return _cache["nc"]

    from contextlib import ExitStack
    import concourse.tile as tile
    from concourse import mybir, bacc
    from concourse.alu_op_type import AluOpType

    fp32, bf16 = mybir.dt.float32, mybir.dt.bfloat16
    fp8 = mybir.dt.float8e4
    AF = mybir.ActivationFunctionType
    AX = mybir.AxisListType
    MUL = AluOpType.mult
    ADD = AluOpType.add

    nc = bacc.Bacc("TRN2", target_bir_lowering=False, debug=False,
                   enable_asserts=False)

    adjt_d = nc.dram_tensor("adjt", [N_NODES, H * QS], fp8, kind="ExternalInput").ap()
    hT_d = nc.dram_tensor("hT", [IN, N_NODES], bf16, kind="ExternalInput").ap()
    hTq_d = nc.dram_tensor("hTq", [IN, QS], bf16, kind="ExternalInput").ap()
    wqp_d = nc.dram_tensor("wqp", [IN, H * DH], bf16, kind="ExternalInput").ap()
    wkp_d = nc.dram_tensor("wkp", [IN, H * DH], bf16, kind="ExternalInput").ap()
    wvp_d = nc.dram_tensor("wvp", [IN, H * DH], bf16, kind="ExternalInput").ap()
    w1_d = nc.dram_tensor("w1", [IN, F1], bf16, kind="ExternalInput").ap()
    w2_d = nc.dram_tensor("w2", [F1, DOUT], bf16, kind="ExternalInput").ap()
    b1_d = nc.dram_tensor("b1", [128, F1 // 128], fp32, kind="ExternalInput").ap()
    b2_d = nc.dram_tensor("b2", [1, DOUT], fp32, kind="ExternalInput").ap()
    out_d = nc.dram_tensor("out", [QS, DOUT], fp32, kind="ExternalOutput").ap()

    with ExitStack() as ctx:
        tc = ctx.enter_context(tile.TileContext(nc))
        pc = ctx.enter_context(tc.tile_pool(name="const", bufs=1))
        pst = ctx.enter_context(tc.tile_pool(name="stp", bufs=2, space="PSUM"))
        pxt = ctx.enter_context(tc.tile_pool(name="xtp", bufs=1, space="PSUM"))
        pa = ctx.enter_context(tc.tile_pool(name="adjp", bufs=4))
        pm = ctx.enter_context(tc.tile_pool(name="mkp", bufs=4))
        ppt = ctx.enter_context(tc.tile_pool(name="ptp", bufs=3))
        psm = ctx.enter_context(tc.tile_pool(name="smallp", bufs=2))

        # ---------------- constants / prep ----------------
        hT_sb = [pc.tile([128, N_NODES], bf16, tag=f"hT{dc}", name=f"hT{dc}") for dc in range(2)]
        for dc in range(2):
            nc.gpsimd.dma_start(out=hT_sb[dc][:], in_=hT_d[dc * 128:(dc + 1) * 128, :])
        hTq_sb = [pc.tile([128, QS], bf16, tag=f"hTq{dc}", name=f"hTq{dc}") for dc in range(2)]
        for dc in range(2):
            nc.gpsimd.dma_start(out=hTq_sb[dc][:], in_=hTq_d[dc * 128:(dc + 1) * 128, :])

        # weight packs: cols dc*256 + (head*64+f)
        wq_sb = pc.tile([128, 2 * H * DH], bf16, tag="wq")
        wk_sb = pc.tile([128, 2 * H * DH], bf16, tag="wk")
        wv_sb = pc.tile([128, 2 * H * DH], bf16, tag="wv")
        for sb, d in ((wq_sb, wqp_d), (wk_sb, wkp_d), (wv_sb, wvp_d)):
            for dc in range(2):
                nc.gpsimd.dma_start(out=sb[:, dc * 256:(dc + 1) * 256],
                                    in_=d[dc * 128:(dc + 1) * 128, :])
        w1_sb = [pc.tile([128, F1], bf16, tag=f"w1_{dc}", name=f"w1_{dc}") for dc in range(2)]
        for dc in range(2):
            nc.gpsimd.dma_start(out=w1_sb[dc][:], in_=w1_d[dc * 128:(dc + 1) * 128, :])
        w2_sb = pc.tile([128, 4 * DOUT], bf16, tag="w2")
        for fc in range(4):
            nc.gpsimd.dma_start(out=w2_sb[:, fc * DOUT:(fc + 1) * DOUT],
                                in_=w2_d[fc * 128:(fc + 1) * 128, :])
        b1_sb = pc.tile([128, F1 // 128], fp32, tag="b1")
        nc.gpsimd.dma_start(out=b1_sb[:], in_=b1_d[:, :])
        b2_sb = pc.tile([1, DOUT], fp32, tag="b2")
        nc.gpsimd.dma_start(out=b2_sb[:], in_=b2_d[:, :])
        b2b_sb = pc.tile([1, DOUT], bf16, tag="b2b")
        nc.vector.tensor_copy(b2b_sb[:], b2_sb[:])
        ones1_sb = pc.tile([1, 128], bf16, tag="ones1")
        nc.vector.memset(ones1_sb[:], 1.0)

        # q^T / k^T: head pairs packed on partitions (pair p -> heads 2p,2p+1)
        # Prep matmuls beyond what attention j-block 0..7 needs are emitted
        # INSIDE the attention loop (prep_chunks) so they fill PE bubbles
        # instead of serializing ~25us of cold-p-state matmuls up front.
        qT_sb = [pc.tile([128, QS], bf16, tag=f"qT{p}", name=f"qT{p}") for p in range(2)]
        kT_sb = [pc.tile([128, N_NODES], bf16, tag=f"kT{p}", name=f"kT{p}") for p in range(2)]
        vp = pc.tile([128, NJB * H, DH + 1], bf16, tag="vp")
        nc.vector.memset(vp[:, :, DH:DH + 1], 1.0)

        def emit_qT(p):
            ps = pst.tile([128, QS], fp32, tag="st", name=f"qTps{p}")
            for dc in range(2):
                nc.tensor.matmul(ps[:],
                                 wq_sb[:, dc * 256 + p * 128: dc * 256 + (p + 1) * 128],
                                 hTq_sb[dc][:],
                                 start=(dc == 0), stop=(dc == 1))
            nc.scalar.copy(qT_sb[p][:], ps[:])

        def emit_kT(p, jt):  # one 1024-wide j chunk of kT for pair p
            ps = pst.tile([128, 2, 512], fp32, tag="st", name=f"kTps{p}_{jt}")
            for half in range(2):
                for dc in range(2):
                    nc.tensor.matmul(
                        ps[:, half, :],
                        wk_sb[:, dc * 256 + p * 128: dc * 256 + (p + 1) * 128],
                        hT_sb[dc][:, jt * 1024 + half * 512: jt * 1024 + (half + 1) * 512],
                        start=(dc == 0), stop=(dc == 1))
            nc.scalar.copy(kT_sb[p][:, jt * 1024:(jt + 1) * 1024], ps[:, :, :])

        def emit_v(jq):  # v (with interleaved ones) for j-blocks 4jq..4jq+3
            ps = pst.tile([128, 16, DH], fp32, tag="st", name=f"vps{jq}")
            for jj in range(4):
                jb = jq * 4 + jj
                for dc in range(2):
                    nc.tensor.matmul(ps[:, jj * 4:(jj + 1) * 4, :],
                                     hT_sb[dc][:, jb * 128:(jb + 1) * 128],
                                     wv_sb[:, dc * 256:(dc + 1) * 256],
                                     start=(dc == 0), stop=(dc == 1))
            nc.scalar.copy(vp[:, jq * 16:(jq + 1) * 16, 0:DH], ps[:, :, :])

        # minimal prep for attention j-blocks 0..7 / PV blocks 0..7
        emit_qT(0)
        emit_qT(1)
        emit_kT(0, 0)
        emit_kT(1, 0)
        emit_v(0)
        emit_v(1)
        prep_chunks = [lambda jt=jt, p=p: emit_kT(p, jt)
                       for jt in range(1, N_NODES // 1024) for p in range(2)]
        prep_chunks += [lambda jq=jq: emit_v(jq) for jq in range(2, NJB // 4)]

        # ---------------- attention ----------------
        embT_sb = [pc.tile([128, QS], bf16, tag=f"embT{p}", name=f"embT{p}") for p in range(2)]
        xaug = [pxt.tile([DH + 1, QS], fp32, tag=f"xt{hd}", name=f"xt{hd}") for hd in range(H)]

        pt_q = []
        for jb in range(NJB + PIPE):
            if jb % 2 == 0 and prep_chunks:
                prep_chunks.pop(0)()
            if jb < NJB:
                aj = pa.tile([128, H * QS], fp8, tag="aj")
                nc.sync.dma_start(out=aj[:],
                                  in_=adjt_d[jb * 128:(jb + 1) * 128, :])
                mk4 = pm.tile([128, H, QS], bf16, tag="mk")
                pt4 = ppt.tile([128, H, QS], bf16, tag="pt")
                for g in range(2):  # head pair
                    st2 = pst.tile([128, 2, 512], fp32, tag="st")
                    for i in range(2):  # head 2g+i from partitions i*64
                        nc.tensor.matmul(
                            st2[:, i, :],
                            kT_sb[g][i * 64:(i + 1) * 64, jb * 128:(jb + 1) * 128],
                            qT_sb[g][i * 64:(i + 1) * 64, :],
                            start=True, stop=True)
                    nc.vector.scalar_tensor_tensor(
                        mk4[:, 2 * g:2 * g + 2, :], st2[:, :, :], SCALE,
                        aj[:, g * 1024:(g + 1) * 1024], MUL, MUL)
                nc.scalar.activation(pt4[:, :, :], mk4[:, :, :], AF.Exp)
                pt_q.append((jb, pt4))
            if jb >= PIPE:
                j2, pt = pt_q.pop(0)
                for hd in range(H):
                    nc.tensor.matmul(xaug[hd][:],
                                     vp[:, j2 * H + hd, 0:DH + 1],
                                     pt[:, hd, :],
                                     start=(j2 == 0), stop=(j2 == NJB - 1))

        # denominators: gather rows 64 of the four xaug tiles into one SBUF
        # tile, one reciprocal over all four, then per-head broadcast+mult.
        den4 = psm.tile([1, H, QS], fp32, tag="den4")
        for hd in range(H):
            nc.vector.tensor_copy(den4[0:1, hd, :], xaug[hd][DH:DH + 1, :])
        den4r = psm.tile([1, H, QS], fp32, tag="den4r")
        nc.vector.reciprocal(den4r[:, :, :], den4[:, :, :])
        for hd in range(H):
            p, off = hd // 2, (hd % 2) * 64
            rbc = psm.tile([128, QS], fp32, tag="rbc")
            nc.gpsimd.partition_broadcast(rbc[:], den4r[0:1, hd, :])
            nc.vector.tensor_tensor(embT_sb[p][off:off + 64, :],
                                    xaug[hd][0:DH, :], rbc[off:off + 64, :], MUL)

        # ---------------- FFN + row softmax ----------------
        p1_sb = pc.tile([128, F1 // 128, QS], bf16, tag="p1")
        for fc in range(F1 // 128):
            ps = pst.tile([128, QS], fp32, tag="st")
            for dc in range(2):
                nc.tensor.matmul(ps[:], w1_sb[dc][:, fc * 128:(fc + 1) * 128],
                                 embT_sb[dc][:], start=(dc == 0), stop=(dc == 1))
            nc.scalar.activation(p1_sb[:, fc, :], ps[:], AF.Relu,
                                 bias=b1_sb[:, fc:fc + 1])
        for qc in range(QS // 128):
            ps2 = pst.tile([128, DOUT], fp32, tag="st")
            for fc in range(F1 // 128):
                nc.tensor.matmul(ps2[:],
                                 p1_sb[:, fc, qc * 128:(qc + 1) * 128],
                                 w2_sb[:, fc * DOUT:(fc + 1) * DOUT],
                                 start=(fc == 0), stop=False)
            # + b2 broadcast to all 128 rows via a rank-1 ones matmul
            nc.tensor.matmul(ps2[:], ones1_sb[0:1, :], b2b_sb[0:1, :],
                             start=False, stop=True)
            mx = psm.tile([128, 1], fp32, tag="mx")
            nc.vector.tensor_reduce(mx[:], ps2[:], axis=AX.X, op=AluOpType.max,
                                    negate=True)
            e = psm.tile([128, DOUT], fp32, tag="e")
            sm = psm.tile([128, 1], fp32, tag="sm")
            nc.scalar.activation(e[:], ps2[:], AF.Exp, bias=mx[:], accum_out=sm[:])
            rc = psm.tile([128, 1], fp32, tag="rc")
            nc.vector.reciprocal(rc[:], sm[:])
            o = psm.tile([128, DOUT], fp32, tag="o")
            nc.vector.tensor_scalar_mul(o[:], e[:], rc[:])
            nc.sync.dma_start(out=out_d[qc * 128:(qc + 1) * 128, :], in_=o[:])

    nc.compile()
    _cache["nc"] = nc
    return nc


def make_in_maps(h, adj, Wq, Wk, Wv, W1, b1, W2, b2):
    bf16 = ml_dtypes.bfloat16
    fp8 = ml_dtypes.float8_e4m3
    h32 = np.asarray(h, np.float32)
    hT = np.ascontiguousarray(h32.T).astype(bf16)
    WqP = np.asarray(Wq, np.float32).transpose(1, 0, 2).reshape(IN, H * DH).astype(bf16)
    WkP = np.asarray(Wk, np.float32).transpose(1, 0, 2).reshape(IN, H * DH).astype(bf16)
    WvP = np.asarray(Wv, np.float32).transpose(1, 0, 2).reshape(IN, H * DH).astype(bf16)
    W1b = np.asarray(W1, np.float32).astype(bf16)
    W2b = np.asarray(W2, np.float32).astype(bf16)
    b1r = np.ascontiguousarray(np.asarray(b1, np.float32).reshape(F1 // 128, 128).T)
    b2r = np.asarray(b2, np.float32).reshape(1, DOUT)
    # adj [H, N, N] fp32 0/1 -> fp8 (exact), then per-core transposed slice
    adj8 = np.asarray(adj, np.float32).astype(fp8)
    # adjT8 [j, hd, q_global]
    adjT8 = np.ascontiguousarray(adj8.transpose(2, 0, 1))
    in_maps = []
    for c in range(NCORES):
        q0 = c * QS
        adjt = np.ascontiguousarray(
            adjT8[:, :, q0:q0 + QS]).reshape(N_NODES, H * QS)
        in_maps.append({
            "adjt": adjt,
            "hT": hT,
            "hTq": np.ascontiguousarray(hT[:, q0:q0 + QS]),
            "wqp": WqP, "wkp": WkP, "wvp": WvP,
            "w1": W1b, "w2": W2b, "b1": b1r, "b2": b2r,
        })
    return in_maps


def kernel(h, adj, Wq, Wk, Wv, W1, b1, W2, b2):
    import os
    nc = build()
    from concourse.bass_utils import run_bass_kernel_spmd
    in_maps = make_in_maps(h, adj, Wq, Wk, Wv, W1, b1, W2, b2)
    trace = bool(os.environ.get("BASS_KERNEL_TRACE"))
    res = run_bass_kernel_spmd(nc, in_maps, list(range(NCORES)), trace=trace)
    if trace and res.exec_time_ns is not None:
        print(f"HW exec time: {res.exec_time_ns} ns")
        kernel.last_exec_time_ns = res.exec_time_ns
    out = np.concatenate([np.asarray(res.results[c]["out"]) for c in range(NCORES)],
                         axis=0)
    return out.astype(np.float32)
